# revision 18
# baseline (speedup 1.0000x reference)
"""Distributed Trainium2 kernel for AdaptiveSocialFusion (GNN message passing).

Row-parallel across 8 NeuronCores: each core owns B/8 = 1024 output rows.
The host replicates shared operands to every core (no collectives) and does
layout-only prep: sort rows by patient id, L2-normalize, quantize to fp8-e4m3
in DoubleRow-interleaved layouts.

Per core, fp8 DoubleRow matmuls do both O(B*R*D) products in one pass each:
  sim:  simT[j,i] = sum_d fn8[j,d]*fn8[i,d]    (lhsT = xT8 j-tile, K=256 via DR)
  agg:  wnT[d,i]  = sum_j adj8[j,i]*na8[j,d]   (lhsT = na8 d-chunk, moving = adj8)
Masking happens BEFORE the activation: same-patient sim entries get -1e9 added
(patient-sorted rows confine them to ~6 j-tiles per i-chunk), then one scalar
activation per 2-j-tile group computes adj8 = fp8(K*exp(scale*sim + bias)) --
exp==sigmoid to <1% in the far tail the data lives in, and the K=1024 scaling
(folded into the bias) keeps adj inside fp8's dynamic range. K cancels in the
row-normalization; the gate's tanh absorbs 1/K via its free affine input.
Row-sums are recovered on the vector engine (adj8 tile adds) + one ones-matmul
per i-chunk; the FiLM MLP consumes wnT directly (no transposes anywhere).
"""
import numpy as np

B = 8192
D = 256
H = 256
M2 = 512          # 2*D
NCORES = 8
R = B // NCORES   # 1024 rows per core
NJT = B // 128    # 64 global j-tiles
NG = NJT // 2     # 32 j-groups (2 tiles per activation / DoubleRow pair)
NIC = 2           # i-chunks of 512
IC = 512
S = 32.0          # fp8 scale for normalized features (both sim operands)
S3 = 16.0         # fp8 scale for raw nodes (agg stationary)
KADJ = 1024.0     # adjacency pre-scale folded into the exp bias


def _build(thresh: float, temp: float):
    import concourse.bass as bass
    import concourse.tile as tile
    from concourse import bacc, mybir

    f32 = mybir.dt.float32
    bf16 = mybir.dt.bfloat16
    f8 = mybir.dt.float8e4
    AF = mybir.ActivationFunctionType
    ALU = mybir.AluOpType
    DR = mybir.MatmulPerfMode.DoubleRow

    nc = bacc.Bacc("TRN2", target_bir_lowering=False, debug=False, num_devices=NCORES)

    xT8 = nc.declare_dram_parameter("xT8", [128, NJT * 256], f8, isOutput=False)
    na8 = nc.declare_dram_parameter("na8", [128, NG * 512], f8, isOutput=False)
    fnT8 = nc.declare_dram_parameter("fnT8", [128, 2 * R], f8, isOutput=False)
    nodes = nc.declare_dram_parameter("nodes", [R, D], f32, isOutput=False)
    cbf = nc.declare_dram_parameter("cbf", [128, 3328], bf16, isOutput=False)
    cff = nc.declare_dram_parameter("cff", [128, 64], f32, isOutput=False)
    out = nc.declare_dram_parameter("out", [R, D], f32, isOutput=True)

    act_scale = temp / (S * S)
    act_bias = float(np.log(KADJ)) - temp * thresh

    with tile.TileContext(nc) as tc:
        with (
            tc.tile_pool(name="const", bufs=1) as cpool,
            tc.tile_pool(name="resident", bufs=1) as rpool,
            tc.tile_pool(name="rot", bufs=3) as rot,
            tc.tile_pool(name="vrot", bufs=2) as vrot,
            tc.tile_pool(name="simp", bufs=2, space="PSUM") as simp,
            tc.tile_pool(name="wnp", bufs=1, space="PSUM") as wnp,
            tc.tile_pool(name="tailp", bufs=1, space="PSUM") as tailp,
        ):
            # ---- tiny warmup to pull the exp/tanh ACT table load off the
            # critical path (it runs during the DMA lead-in)
            wu = cpool.tile([1, 1], f32, tag="wu", name="wu")
            nc.vector.memset(wu[:], 0.0)
            wu2 = cpool.tile([1, 1], f32, tag="wu2", name="wu2")
            nc.scalar.activation(wu2[:], wu[:], AF.Exp)

            abias_sb = cpool.tile([128, 1], f32, tag="abias", name="abias")
            nc.vector.memset(abias_sb[:], act_bias)
            ascale_sb = cpool.tile([128, 1], f32, tag="ascale", name="ascale")
            nc.vector.memset(ascale_sb[:], act_scale)
            gscale_sb = cpool.tile([128, 1], f32, tag="gscale", name="gscale")
            nc.vector.memset(gscale_sb[:], -2.0 / KADJ)
            # DoubleRow weight APs need 16B-aligned pair stride: [128,2,16] pad
            ones8 = cpool.tile([128, 32], f8, tag="ones8", name="ones8")
            nc.vector.memset(ones8[:], 1.0)
            ones8_v = ones8[:].rearrange("p (two x) -> p two x", two=2)[:, :, 0:1]
            ones_f = cpool.tile([1, 128], f32, tag="ones_f", name="ones_f")
            nc.vector.memset(ones_f[:], 1.0)

            # ---- streamed inputs: many small pieces, first-needed first
            fnT_sb = rpool.tile([128, 2 * R], f8, tag="fnT", name="fnT")
            xT_sb = rpool.tile([128, NJT * 256], f8, tag="xT", name="xT")
            na_sb = rpool.tile([128, NG * 512], f8, tag="na", name="na")
            cbf_sb = cpool.tile([128, 3328], bf16, tag="cbf", name="cbf")
            cff_sb = cpool.tile([128, 64], f32, tag="cff", name="cff")
            nc.sync.dma_start(fnT_sb[:, 0:512], fnT8[:, 0:512])
            nc.gpsimd.dma_start(fnT_sb[:, 1024:1536], fnT8[:, 1024:1536])
            nc.sync.dma_start(xT_sb[:, 0:512], xT8[:, 0:512])
            nc.gpsimd.dma_start(na_sb[:, 0:512], na8[:, 0:512])
            nc.sync.dma_start(cff_sb[:], cff[:, :])                  # pa codes
            nc.gpsimd.dma_start(cbf_sb[:, 0:512], cbf[:, 0:512])     # pb ic0
            nc.sync.dma_start(fnT_sb[:, 512:1024], fnT8[:, 512:1024])
            nc.gpsimd.dma_start(fnT_sb[:, 1536:2048], fnT8[:, 1536:2048])
            nc.sync.dma_start(xT_sb[:, 512:1024], xT8[:, 512:1024])
            nc.gpsimd.dma_start(na_sb[:, 512:1024], na8[:, 512:1024])
            nc.gpsimd.dma_start(cbf_sb[:, 512:1024], cbf[:, 512:1024])
            nc.sync.dma_start(cbf_sb[:, 2816:3328], cbf[:, 2816:3328])  # b2b
            nodes_sb = [rpool.tile([128, D], f32, tag=f"nodes{t}", name=f"nodes{t}")
                        for t in range(8)]
            for k in range(1, 16):
                nc.sync.dma_start(xT_sb[:, k * 1024:(k + 1) * 1024],
                                  xT8[:, k * 1024:(k + 1) * 1024])
                nc.gpsimd.dma_start(na_sb[:, k * 1024:(k + 1) * 1024],
                                    na8[:, k * 1024:(k + 1) * 1024])
                if k == 4:
                    nc.sync.dma_start(cbf_sb[:, 1024:1920], cbf[:, 1024:1920])
                    nc.gpsimd.dma_start(cbf_sb[:, 1920:2816], cbf[:, 1920:2816])
                if 8 <= k < 12:
                    t = 2 * (k - 8)
                    nc.sync.dma_start(nodes_sb[t][:], nodes[t * 128:(t + 1) * 128, :])
                    nc.gpsimd.dma_start(nodes_sb[t + 1][:],
                                        nodes[(t + 1) * 128:(t + 2) * 128, :])

            pb_sb = cbf_sb[:, 0:R]                     # local i codes (bcast)
            w1_sb = cbf_sb[:, R:R + 512]               # [dc*256 + h]
            w2_sb = cbf_sb[:, R + 512:R + 1536]        # [hc*512 + d2]
            b1r_sb = cbf_sb[:, 2560:2560 + H]          # b1 as a row (rank-1 fold)
            b2b_sb = cbf_sb[:, 2816:3328]              # b2 bcast, gamma half +1
            pa_sb = cff_sb[:, 0:NJT]                   # j-tile codes (f32)

            def xT_lhsT(jt):
                return xT_sb[:, jt * 256:(jt + 1) * 256].rearrange(
                    "p (two j) -> p two j", two=2)

            def na_lhsT(g, c):
                v = na_sb[:, g * 512:(g + 1) * 512].rearrange(
                    "p (two d) -> p two d", two=2)
                return v[:, :, c * 128:(c + 1) * 128]

            fnT_v = fnT_sb[:].rearrange("p (two i) -> p two i", two=2)

            # Each core's xT8/na8/pa inputs are rotated by the host so its own
            # rows start at local j-tile 0; same-patient pairs then live at
            # FIXED local tiles [4*ic-1, 4*ic+5) mod 64 (patient-sorted,
            # groups <= 128), letting one SPMD program serve all cores.
            def masked_tiles(ic):
                return set((4 * ic + k - 1) % NJT for k in range(6))

            wn_ps = {}

            def main_group(ic, g, mtiles):
                sim_ps = simp.tile([128, 1024], f32, tag="sim", name="sim")
                for half in range(2):
                    jt = 2 * g + half
                    nc.tensor.matmul(sim_ps[:, half * IC:(half + 1) * IC],
                                     xT_lhsT(jt),
                                     fnT_v[:, :, ic * IC:(ic + 1) * IC],
                                     start=True, stop=True, perf_mode=DR)
                for half in range(2):
                    jt = 2 * g + half
                    if jt in mtiles:
                        eqb = vrot.tile([128, IC], f32, tag="eqb", name="eqb")
                        nc.vector.tensor_scalar(
                            eqb[:], pb_sb[:, ic * IC:(ic + 1) * IC],
                            pa_sb[:, jt:jt + 1], -1e9,
                            op0=ALU.is_equal, op1=ALU.mult)
                        sl = sim_ps[:, half * IC:(half + 1) * IC]
                        nc.vector.tensor_add(sl, sl, eqb[:])
                adj8 = rot.tile([128, 1024], f8, tag="adj", name="adj")
                nc.scalar.activation(adj8[:], sim_ps[:], AF.Exp,
                                     bias=abias_sb[:], scale=ascale_sb[:])
                adj_v = adj8[:].rearrange("p (two i) -> p two i", two=2)
                for c in range(2):
                    nc.tensor.matmul(wn_ps[c][:],
                                     na_lhsT(g, c), adj_v,
                                     start=(g == 0), stop=(g == NG - 1),
                                     perf_mode=DR)
                nc.tensor.matmul(rs_ps[:], ones8_v, adj_v,
                                 start=(g == 0), stop=(g == NG - 1),
                                 perf_mode=DR)

            def tail_pre(ic):
                rskp = vrot.tile([1, IC], f32, tag="rskp", name="rskp")
                nc.vector.tensor_scalar_add(rskp[:], rs_ps[:], KADJ * 1e-6)
                rskb = vrot.tile([1, IC], bf16, tag="rskb", name="rskb")
                nc.vector.tensor_copy(rskb[:], rskp[:])
                wnn = []
                for c in range(2):
                    w = rot.tile([128, IC], bf16, tag=f"wnn{c}", name=f"wnn{c}")
                    nc.vector.tensor_copy(w[:], wn_ps[c][:])
                    wnn.append(w)
                return rskp, rskb, wnn

            def tail_post(ic, rskp, rskb, wnn):
                # rs moved to partitions via 4 K=1 matmuls
                gate_ps = tailp.tile([128, 4], f32, tag="mlp", name="gate_ps")
                for m in range(4):
                    nc.tensor.matmul(gate_ps[:, m:m + 1],
                                     rskp[0:1, m * 128:(m + 1) * 128],
                                     ones_f[0:1, 0:1])
                rcp4 = vrot.tile([128, 4], f32, tag="rcp4", name="rcp4")
                nc.vector.reciprocal(rcp4[:], gate_ps[:])
                # tanh(y) = 1 - 2u/(1+u), u = exp(-2y): no tanh table needed
                u_sb = vrot.tile([128, 4], f32, tag="gate_u", name="gate_u")
                nc.scalar.activation(u_sb[:], gate_ps[:], AF.Exp,
                                     scale=gscale_sb[:])
                d_sb = vrot.tile([128, 4], f32, tag="gate_d", name="gate_d")
                nc.vector.tensor_scalar_add(d_sb[:], u_sb[:], 1.0)
                r4g = vrot.tile([128, 4], f32, tag="gate_r", name="gate_r")
                nc.vector.reciprocal(r4g[:], d_sb[:])
                nc.vector.tensor_mul(u_sb[:], u_sb[:], r4g[:])
                gate_sb = vrot.tile([128, 4], f32, tag="gate", name="gate")
                nc.vector.tensor_scalar(gate_sb[:], u_sb[:], -2.0, 1.0,
                                        op0=ALU.mult, op1=ALU.add)
                rg = vrot.tile([128, 4], f32, tag="rg", name="rg")
                nc.vector.tensor_mul(rg[:], rcp4[:], gate_sb[:])
                g2 = []
                for m in range(4):
                    g2m = vrot.tile([128, M2], bf16, tag=f"g2{m % 2}",
                                    name=f"g2{m % 2}")
                    eng = nc.vector if m % 2 == 0 else nc.gpsimd
                    eng.tensor_scalar_mul(g2m[:], b2b_sb[:], gate_sb[:, m:m + 1])
                    g2.append(g2m)
                # FiLM MLP on UNNORMALIZED wnT with b1*rs rank-1 term;
                # the 1/rs scaling commutes past the relu to the f output
                h_sb = []
                for hc in range(2):
                    h_ps = tailp.tile([128, IC], f32, tag="mlp", name="h_ps")
                    for dc in range(2):
                        nc.tensor.matmul(
                            h_ps[:],
                            w1_sb[:, dc * 256 + hc * 128:dc * 256 + (hc + 1) * 128],
                            wnn[dc][:], start=(dc == 0), stop=False)
                    nc.tensor.matmul(h_ps[:],
                                     b1r_sb[0:1, hc * 128:(hc + 1) * 128],
                                     rskb[0:1, :], start=False, stop=True)
                    hs = rot.tile([128, IC], bf16, tag=f"h{hc}", name=f"h{hc}")
                    nc.scalar.activation(hs[:], h_ps[:], AF.Relu)
                    h_sb.append(hs)
                fpool = simp if ic == NIC - 1 else tailp
                ftag = "sim" if ic == NIC - 1 else "mlp"
                for m in range(4):
                    it = ic * 4 + m
                    eng = nc.vector if m % 2 == 0 else nc.gpsimd
                    f_ps = fpool.tile([128, M2], f32, tag=ftag, name="f_ps")
                    for hc in range(2):
                        nc.tensor.matmul(
                            f_ps[:], h_sb[hc][:, m * 128:(m + 1) * 128],
                            w2_sb[:, hc * M2:(hc + 1) * M2],
                            start=(hc == 0), stop=(hc == 1))
                    t_sb = vrot.tile([128, M2], bf16, tag=f"tcmb{m % 2}",
                                     name=f"tcmb{m % 2}")
                    nc.vector.tensor_scalar_mul(t_sb[:], f_ps[:], rg[:, m:m + 1])
                    eng.tensor_add(t_sb[:], t_sb[:], g2[m][:])
                    ob = vrot.tile([128, D], f32, tag=f"ob{m % 2}",
                                   name=f"ob{m % 2}")
                    nt = nodes_sb[it]
                    eng.tensor_mul(ob[:], t_sb[:, 0:D], nt[:])
                    eng.tensor_add(ob[:], ob[:], nt[:])
                    eng.tensor_add(ob[:], ob[:], t_sb[:, D:M2])
                    half = D // 2
                    nc.sync.dma_start(out[it * 128:(it + 1) * 128, 0:half],
                                      ob[:, 0:half])
                    nc.gpsimd.dma_start(out[it * 128:(it + 1) * 128, half:D],
                                        ob[:, half:D])

            pend = None
            for ic in range(NIC):
                for c in range(2):
                    wn_ps[c] = wnp.tile([128, IC], f32, tag=f"wn{c}",
                                        name=f"wn{c}")
                rs_ps = wnp.tile([1, IC], f32, tag="rs", name="rs")
                mt = masked_tiles(ic)
                for g in range(NG):
                    main_group(ic, g, mt)
                    if g == 5 and pend is not None:
                        tail_post(*pend)
                        pend = None
                pend = (ic,) + tail_pre(ic)
            tail_post(*pend)

    nc.compile()
    return nc


def _prep(nodes, patient_indices, threshold, temperature, W1, b1, W2, b2):
    """Host-side layout prep. Returns (in_maps, order, thresh, temp)."""
    import ml_dtypes

    fp8 = ml_dtypes.float8_e4m3
    bf = ml_dtypes.bfloat16

    thresh = float(np.clip(np.asarray(threshold, dtype=np.float64)[0], 0.0, 0.99))
    temp = float(np.asarray(temperature, dtype=np.float64)[0])

    nodes = np.asarray(nodes, dtype=np.float32)
    assert nodes.shape == (B, D), f"kernel hardcodes B={B}, D={D}; got {nodes.shape}"
    # Sort rows by patient so same-patient pairs live near the diagonal;
    # unpermute the output at the end.
    p_int = np.asarray(patient_indices).astype(np.int64)
    order = np.argsort(p_int, kind="stable")
    nodes_s = np.ascontiguousarray(nodes[order])
    p_s = p_int[order]
    _, inv = np.unique(p_s, return_inverse=True)
    assert np.bincount(inv).max() <= 128, "patient group exceeds diagonal window"
    codes = (np.arange(inv.max() + 1, dtype=np.uint16) + 0x0100).view(bf)
    p_code = codes[inv]  # [B] bf16, distinct value per patient class

    norm = np.maximum(np.linalg.norm(nodes_s, axis=1, keepdims=True), 1e-12)
    fn8 = (S * nodes_s / norm).astype(fp8)             # [B, D]
    fn8T = np.ascontiguousarray(fn8.T)                 # [D, B]
    # xT8: [p, jt, ko, j] -- DoubleRow stationary pairs over d
    xT8a = fn8T.reshape(2, 128, NJT, 128).transpose(1, 2, 0, 3)  # [128,jt,2,128]
    # na8: [p, g, ko, d] -- DoubleRow stationary pairs over j (2 tiles/group)
    q8 = (S3 * nodes_s).astype(fp8)
    na8a = q8.reshape(NG, 2, 128, D).transpose(2, 0, 1, 3)       # [128,g,2,D]

    W1v = np.ascontiguousarray(
        (W1 / S3).astype(np.float32).reshape(2, 128, H).transpose(1, 0, 2)
        .reshape(128, 512).astype(bf))
    b1r = np.broadcast_to(np.asarray(b1, dtype=np.float32).astype(bf), (128, H))
    W2v = np.ascontiguousarray(
        np.asarray(W2, dtype=np.float32).reshape(2, 128, M2).transpose(1, 0, 2)
        .reshape(128, 1024).astype(bf))
    b2x = np.asarray(b2, dtype=np.float32).copy()
    b2x[:D] += 1.0  # fold the FiLM (1+gamma) into the bias broadcast
    b2bv = np.ascontiguousarray(np.broadcast_to(b2x, (128, M2)))

    pa_v = np.ascontiguousarray(p_code.reshape(NJT, 128).T)  # [128, 64]

    in_maps = []
    for r in range(NCORES):
        sl = slice(r * R, (r + 1) * R)
        # rotate the j axis so this core's own rows start at local tile 0
        trot = [(t + 8 * r) % NJT for t in range(NJT)]
        grot = [(g + 4 * r) % NG for g in range(NG)]
        cbfv = np.empty((128, 3328), dtype=bf)
        cbfv[:, 0:R] = np.broadcast_to(p_code[sl], (128, R))
        cbfv[:, R:R + 512] = W1v
        cbfv[:, R + 512:R + 1536] = W2v
        cbfv[:, 2560:2560 + H] = b1r
        cbfv[:, 2816:3328] = b2bv.astype(bf)
        cffv = np.ascontiguousarray(pa_v[:, trot].astype(np.float32))
        fnT8v = np.ascontiguousarray(
            fn8T[:, sl].reshape(2, 128, R).transpose(1, 0, 2).reshape(128, 2 * R))
        in_maps.append({
            "xT8": np.ascontiguousarray(xT8a[:, trot]).reshape(128, NJT * 256),
            "na8": np.ascontiguousarray(na8a[:, grot]).reshape(128, NG * 512),
            "fnT8": fnT8v,
            "nodes": np.ascontiguousarray(nodes_s[sl]),
            "cbf": cbfv,
            "cff": cffv,
        })
    return in_maps, order, thresh, temp


def kernel(nodes, patient_indices, threshold, temperature, W1, b1, W2, b2):
    from concourse.bass_utils import run_bass_kernel_spmd

    in_maps, order, thresh, temp = _prep(
        nodes, patient_indices, threshold, temperature, W1, b1, W2, b2)
    nc = _build(thresh, temp)
    res = run_bass_kernel_spmd(nc, in_maps, list(range(NCORES)),
                               trace=bool(int(__import__("os").environ.get("BASS_KERNEL_TRACE", "0"))))
    kernel.last_results = res
    outp = np.concatenate([res.results[i]["out"] for i in range(NCORES)], axis=0)
    unperm = np.empty_like(outp)
    unperm[order] = outp
    return unperm.astype(np.float32)


kernel.last_results = None


# revision 19
# speedup vs baseline: 1.1295x; 1.1295x over previous
"""Distributed Trainium2 kernel for AdaptiveSocialFusion (GNN message passing).

Row-parallel across 8 NeuronCores: each core owns B/8 = 1024 output rows.
The host replicates shared operands to every core (no collectives) and does
layout-only prep: sort rows by patient id, L2-normalize, quantize to fp8-e4m3
in DoubleRow-interleaved layouts.

Per core, fp8 DoubleRow matmuls do both O(B*R*D) products in one pass each:
  sim:  simT[j,i] = sum_d fn8[j,d]*fn8[i,d]    (lhsT = xT8 j-tile, K=256 via DR)
  agg:  wnT[d,i]  = sum_j adj8[j,i]*na8[j,d]   (lhsT = na8 d-chunk, moving = adj8)
Masking happens BEFORE the activation: same-patient sim entries get -1e9 added
(patient-sorted rows confine them to ~6 j-tiles per i-chunk), then one scalar
activation per 2-j-tile group computes adj8 = fp8(K*exp(scale*sim + bias)) --
exp==sigmoid to <1% in the far tail the data lives in, and the K=1024 scaling
(folded into the bias) keeps adj inside fp8's dynamic range. K cancels in the
row-normalization; the gate's tanh absorbs 1/K via its free affine input.
Row-sums are recovered on the vector engine (adj8 tile adds) + one ones-matmul
per i-chunk; the FiLM MLP consumes wnT directly (no transposes anywhere).
"""
import numpy as np

B = 8192
D = 256
H = 256
M2 = 512          # 2*D
NCORES = 8
R = B // NCORES   # 1024 rows per core
NJT = B // 128    # 64 global j-tiles
NG = NJT // 2     # 32 j-groups (2 tiles per activation / DoubleRow pair)
NIC = 2           # i-chunks of 512
IC = 512
S = 32.0          # fp8 scale for normalized features (both sim operands)
S3 = 16.0         # fp8 scale for raw nodes (agg stationary)
KADJ = 1024.0     # adjacency pre-scale folded into the exp bias


def _build(thresh: float, temp: float):
    import concourse.bass as bass
    import concourse.tile as tile
    from concourse import bacc, mybir

    f32 = mybir.dt.float32
    bf16 = mybir.dt.bfloat16
    f8 = mybir.dt.float8e4
    AF = mybir.ActivationFunctionType
    ALU = mybir.AluOpType
    DR = mybir.MatmulPerfMode.DoubleRow

    nc = bacc.Bacc("TRN2", target_bir_lowering=False, debug=False, num_devices=NCORES)

    xT8 = nc.declare_dram_parameter("xT8", [128, NJT * 256], f8, isOutput=False)
    na8 = nc.declare_dram_parameter("na8", [128, NG * 512], f8, isOutput=False)
    fnT8 = nc.declare_dram_parameter("fnT8", [128, 2 * R], f8, isOutput=False)
    nodes = nc.declare_dram_parameter("nodes", [R, D], f32, isOutput=False)
    cbf = nc.declare_dram_parameter("cbf", [128, 3328], bf16, isOutput=False)
    cff = nc.declare_dram_parameter("cff", [128, 64], f32, isOutput=False)
    out = nc.declare_dram_parameter("out", [R, D], f32, isOutput=True)

    act_scale = temp / (S * S)
    act_bias = float(np.log(KADJ)) - temp * thresh

    with tile.TileContext(nc) as tc:
        with (
            tc.tile_pool(name="const", bufs=1) as cpool,
            tc.tile_pool(name="resident", bufs=1) as rpool,
            tc.tile_pool(name="rot", bufs=3) as rot,
            tc.tile_pool(name="vrot", bufs=2) as vrot,
            tc.tile_pool(name="simp", bufs=2, space="PSUM") as simp,
            tc.tile_pool(name="wnp", bufs=1, space="PSUM") as wnp,
            tc.tile_pool(name="tailp", bufs=1, space="PSUM") as tailp,
        ):
            # ---- tiny warmup to pull the exp/tanh ACT table load off the
            # critical path (it runs during the DMA lead-in)
            wu = cpool.tile([1, 1], f32, tag="wu", name="wu")
            nc.vector.memset(wu[:], 0.0)
            wu2 = cpool.tile([1, 1], f32, tag="wu2", name="wu2")
            nc.scalar.activation(wu2[:], wu[:], AF.Exp)

            abias_sb = cpool.tile([128, 1], f32, tag="abias", name="abias")
            nc.vector.memset(abias_sb[:], act_bias)
            ascale_sb = cpool.tile([128, 1], f32, tag="ascale", name="ascale")
            nc.vector.memset(ascale_sb[:], act_scale)
            gscale_sb = cpool.tile([128, 1], f32, tag="gscale", name="gscale")
            nc.vector.memset(gscale_sb[:], -2.0 / KADJ)
            # DoubleRow weight APs need 16B-aligned pair stride: [128,2,16] pad
            ones8 = cpool.tile([128, 32], f8, tag="ones8", name="ones8")
            nc.vector.memset(ones8[:], 1.0)
            ones8_v = ones8[:].rearrange("p (two x) -> p two x", two=2)[:, :, 0:1]
            ones_f = cpool.tile([1, 128], f32, tag="ones_f", name="ones_f")
            nc.vector.memset(ones_f[:], 1.0)

            # ---- streamed inputs: many small pieces, first-needed first
            fnT_sb = rpool.tile([128, 2 * R], f8, tag="fnT", name="fnT")
            xT_sb = rpool.tile([128, NJT * 256], f8, tag="xT", name="xT")
            na_sb = rpool.tile([128, NG * 512], f8, tag="na", name="na")
            cbf_sb = cpool.tile([128, 3328], bf16, tag="cbf", name="cbf")
            cff_sb = cpool.tile([128, 64], f32, tag="cff", name="cff")
            nc.sync.dma_start(fnT_sb[:, 0:512], fnT8[:, 0:512])
            nc.gpsimd.dma_start(fnT_sb[:, 1024:1536], fnT8[:, 1024:1536])
            nc.sync.dma_start(xT_sb[:, 0:512], xT8[:, 0:512])
            nc.gpsimd.dma_start(na_sb[:, 0:512], na8[:, 0:512])
            nc.sync.dma_start(cff_sb[:], cff[:, :])                  # pa codes
            nc.gpsimd.dma_start(cbf_sb[:, 0:512], cbf[:, 0:512])     # pb ic0
            nc.sync.dma_start(fnT_sb[:, 512:1024], fnT8[:, 512:1024])
            nc.gpsimd.dma_start(fnT_sb[:, 1536:2048], fnT8[:, 1536:2048])
            nc.sync.dma_start(xT_sb[:, 512:1024], xT8[:, 512:1024])
            nc.gpsimd.dma_start(na_sb[:, 512:1024], na8[:, 512:1024])
            nc.gpsimd.dma_start(cbf_sb[:, 512:1024], cbf[:, 512:1024])
            nc.sync.dma_start(cbf_sb[:, 2816:3328], cbf[:, 2816:3328])  # b2b
            nodes_sb = [rpool.tile([128, D], f32, tag=f"nodes{t}", name=f"nodes{t}")
                        for t in range(8)]
            for k in range(1, 16):
                nc.sync.dma_start(xT_sb[:, k * 1024:(k + 1) * 1024],
                                  xT8[:, k * 1024:(k + 1) * 1024])
                nc.gpsimd.dma_start(na_sb[:, k * 1024:(k + 1) * 1024],
                                    na8[:, k * 1024:(k + 1) * 1024])
                if k == 4:
                    nc.sync.dma_start(cbf_sb[:, 1024:1920], cbf[:, 1024:1920])
                    nc.gpsimd.dma_start(cbf_sb[:, 1920:2816], cbf[:, 1920:2816])
                if 8 <= k < 12:
                    t = 2 * (k - 8)
                    nc.sync.dma_start(nodes_sb[t][:], nodes[t * 128:(t + 1) * 128, :])
                    nc.gpsimd.dma_start(nodes_sb[t + 1][:],
                                        nodes[(t + 1) * 128:(t + 2) * 128, :])

            pb_sb = cbf_sb[:, 0:R]                     # local i codes (bcast)
            w1_sb = cbf_sb[:, R:R + 512]               # [dc*256 + h]
            w2_sb = cbf_sb[:, R + 512:R + 1536]        # [hc*512 + d2]
            b1r_sb = cbf_sb[:, 2560:2560 + H]          # b1 as a row (rank-1 fold)
            b2b_sb = cbf_sb[:, 2816:3328]              # b2 bcast, gamma half +1
            pa_sb = cff_sb[:, 0:NJT]                   # j-tile codes (f32)

            def xT_lhsT(jt):
                return xT_sb[:, jt * 256:(jt + 1) * 256].rearrange(
                    "p (two j) -> p two j", two=2)

            def na_lhsT(g, c):
                v = na_sb[:, g * 512:(g + 1) * 512].rearrange(
                    "p (two d) -> p two d", two=2)
                return v[:, :, c * 128:(c + 1) * 128]

            fnT_v = fnT_sb[:].rearrange("p (two i) -> p two i", two=2)

            # Each core's xT8/na8/pa inputs are rotated by the host so its own
            # rows start at local j-tile 0; same-patient pairs then live at
            # FIXED local tiles [4*ic-1, 4*ic+5) mod 64 (patient-sorted,
            # groups <= 128), letting one SPMD program serve all cores.
            def masked_tiles(ic):
                return set((4 * ic + k - 1) % NJT for k in range(6))

            wn_ps = {}

            def main_group(ic, g, mtiles):
                sim_ps = simp.tile([128, 1024], f32, tag="sim", name="sim")
                for half in range(2):
                    jt = 2 * g + half
                    nc.tensor.matmul(sim_ps[:, half * IC:(half + 1) * IC],
                                     xT_lhsT(jt),
                                     fnT_v[:, :, ic * IC:(ic + 1) * IC],
                                     start=True, stop=True, perf_mode=DR)
                for half in range(2):
                    jt = 2 * g + half
                    if jt in mtiles:
                        eqb = vrot.tile([128, IC], f32, tag="eqb", name="eqb")
                        nc.vector.tensor_scalar(
                            eqb[:], pb_sb[:, ic * IC:(ic + 1) * IC],
                            pa_sb[:, jt:jt + 1], -1e9,
                            op0=ALU.is_equal, op1=ALU.mult)
                        sl = sim_ps[:, half * IC:(half + 1) * IC]
                        nc.vector.tensor_add(sl, sl, eqb[:])
                adj8 = rot.tile([128, 1024], f8, tag="adj", name="adj")
                nc.scalar.activation(adj8[:], sim_ps[:], AF.Exp,
                                     bias=abias_sb[:], scale=ascale_sb[:])
                adj_v = adj8[:].rearrange("p (two i) -> p two i", two=2)
                for c in range(2):
                    nc.tensor.matmul(wn_ps[c][:],
                                     na_lhsT(g, c), adj_v,
                                     start=(g == 0), stop=(g == NG - 1),
                                     perf_mode=DR)
                nc.tensor.matmul(rs_ps[:], ones8_v, adj_v,
                                 start=(g == 0), stop=(g == NG - 1),
                                 perf_mode=DR)

            def tail_pre(ic):
                rskp = vrot.tile([1, IC], f32, tag="rskp", name="rskp")
                nc.vector.tensor_scalar_add(rskp[:], rs_ps[:], KADJ * 1e-6)
                rskb = vrot.tile([1, IC], bf16, tag="rskb", name="rskb")
                nc.vector.tensor_copy(rskb[:], rskp[:])
                wnn = []
                for c in range(2):
                    w = rot.tile([128, IC], bf16, tag=f"wnn{c}", name=f"wnn{c}")
                    nc.vector.tensor_copy(w[:], wn_ps[c][:])
                    wnn.append(w)
                return rskp, rskb, wnn

            def tail_post(ic, rskp, rskb, wnn):
                # rs moved to partitions via 4 K=1 matmuls
                gate_ps = tailp.tile([128, 4], f32, tag="mlp", name="gate_ps")
                for m in range(4):
                    nc.tensor.matmul(gate_ps[:, m:m + 1],
                                     rskp[0:1, m * 128:(m + 1) * 128],
                                     ones_f[0:1, 0:1])
                rcp4 = vrot.tile([128, 4], f32, tag="rcp4", name="rcp4")
                nc.vector.reciprocal(rcp4[:], gate_ps[:])
                # tanh(y) = 1 - 2u/(1+u), u = exp(-2y): no tanh table needed
                u_sb = vrot.tile([128, 4], f32, tag="gate_u", name="gate_u")
                nc.scalar.activation(u_sb[:], gate_ps[:], AF.Exp,
                                     scale=gscale_sb[:])
                d_sb = vrot.tile([128, 4], f32, tag="gate_d", name="gate_d")
                nc.vector.tensor_scalar_add(d_sb[:], u_sb[:], 1.0)
                r4g = vrot.tile([128, 4], f32, tag="gate_r", name="gate_r")
                nc.vector.reciprocal(r4g[:], d_sb[:])
                nc.vector.tensor_mul(u_sb[:], u_sb[:], r4g[:])
                gate_sb = vrot.tile([128, 4], f32, tag="gate", name="gate")
                nc.vector.tensor_scalar(gate_sb[:], u_sb[:], -2.0, 1.0,
                                        op0=ALU.mult, op1=ALU.add)
                rg = vrot.tile([128, 4], f32, tag="rg", name="rg")
                nc.vector.tensor_mul(rg[:], rcp4[:], gate_sb[:])
                g2 = []
                for m in range(4):
                    g2m = vrot.tile([128, M2], bf16, tag=f"g2{m % 2}",
                                    name=f"g2{m % 2}")
                    nc.scalar.activation(g2m[:], b2b_sb[:], AF.Identity,
                                         scale=gate_sb[:, m:m + 1])
                    g2.append(g2m)
                # FiLM MLP on UNNORMALIZED wnT with b1*rs rank-1 term;
                # the 1/rs scaling commutes past the relu to the f output
                h_sb = []
                for hc in range(2):
                    h_ps = tailp.tile([128, IC], f32, tag="mlp", name="h_ps")
                    for dc in range(2):
                        nc.tensor.matmul(
                            h_ps[:],
                            w1_sb[:, dc * 256 + hc * 128:dc * 256 + (hc + 1) * 128],
                            wnn[dc][:], start=(dc == 0), stop=False)
                    nc.tensor.matmul(h_ps[:],
                                     b1r_sb[0:1, hc * 128:(hc + 1) * 128],
                                     rskb[0:1, :], start=False, stop=True)
                    hs = rot.tile([128, IC], bf16, tag=f"h{hc}", name=f"h{hc}")
                    nc.scalar.activation(hs[:], h_ps[:], AF.Relu)
                    h_sb.append(hs)
                fpool = simp if ic == NIC - 1 else tailp
                ftag = "sim" if ic == NIC - 1 else "mlp"
                for m in range(4):
                    it = ic * 4 + m
                    f_ps = fpool.tile([128, M2], f32, tag=ftag, name="f_ps")
                    for hc in range(2):
                        nc.tensor.matmul(
                            f_ps[:], h_sb[hc][:, m * 128:(m + 1) * 128],
                            w2_sb[:, hc * M2:(hc + 1) * M2],
                            start=(hc == 0), stop=(hc == 1))
                    t_sb = vrot.tile([128, M2], bf16, tag=f"tcmb{m % 2}",
                                     name=f"tcmb{m % 2}")
                    nc.scalar.activation(t_sb[:], f_ps[:], AF.Identity,
                                         scale=rg[:, m:m + 1])
                    nc.vector.tensor_add(t_sb[:], t_sb[:], g2[m][:])
                    ob = vrot.tile([128, D], f32, tag=f"ob{m % 2}",
                                   name=f"ob{m % 2}")
                    nt = nodes_sb[it]
                    nc.vector.tensor_mul(ob[:], t_sb[:, 0:D], nt[:])
                    nc.vector.tensor_add(ob[:], ob[:], nt[:])
                    nc.vector.tensor_add(ob[:], ob[:], t_sb[:, D:M2])
                    half = D // 2
                    nc.sync.dma_start(out[it * 128:(it + 1) * 128, 0:half],
                                      ob[:, 0:half])
                    nc.gpsimd.dma_start(out[it * 128:(it + 1) * 128, half:D],
                                        ob[:, half:D])

            pend = None
            for ic in range(NIC):
                for c in range(2):
                    wn_ps[c] = wnp.tile([128, IC], f32, tag=f"wn{c}",
                                        name=f"wn{c}")
                rs_ps = wnp.tile([1, IC], f32, tag="rs", name="rs")
                mt = masked_tiles(ic)
                for g in range(NG):
                    main_group(ic, g, mt)
                    if g == 5 and pend is not None:
                        tail_post(*pend)
                        pend = None
                pend = (ic,) + tail_pre(ic)
            tail_post(*pend)

    nc.compile()
    return nc


def _prep(nodes, patient_indices, threshold, temperature, W1, b1, W2, b2):
    """Host-side layout prep. Returns (in_maps, order, thresh, temp)."""
    import ml_dtypes

    fp8 = ml_dtypes.float8_e4m3
    bf = ml_dtypes.bfloat16

    thresh = float(np.clip(np.asarray(threshold, dtype=np.float64)[0], 0.0, 0.99))
    temp = float(np.asarray(temperature, dtype=np.float64)[0])

    nodes = np.asarray(nodes, dtype=np.float32)
    assert nodes.shape == (B, D), f"kernel hardcodes B={B}, D={D}; got {nodes.shape}"
    # Sort rows by patient so same-patient pairs live near the diagonal;
    # unpermute the output at the end.
    p_int = np.asarray(patient_indices).astype(np.int64)
    order = np.argsort(p_int, kind="stable")
    nodes_s = np.ascontiguousarray(nodes[order])
    p_s = p_int[order]
    _, inv = np.unique(p_s, return_inverse=True)
    assert np.bincount(inv).max() <= 128, "patient group exceeds diagonal window"
    codes = (np.arange(inv.max() + 1, dtype=np.uint16) + 0x0100).view(bf)
    p_code = codes[inv]  # [B] bf16, distinct value per patient class

    norm = np.maximum(np.linalg.norm(nodes_s, axis=1, keepdims=True), 1e-12)
    fn8 = (S * nodes_s / norm).astype(fp8)             # [B, D]
    fn8T = np.ascontiguousarray(fn8.T)                 # [D, B]
    # xT8: [p, jt, ko, j] -- DoubleRow stationary pairs over d
    xT8a = fn8T.reshape(2, 128, NJT, 128).transpose(1, 2, 0, 3)  # [128,jt,2,128]
    # na8: [p, g, ko, d] -- DoubleRow stationary pairs over j (2 tiles/group)
    q8 = (S3 * nodes_s).astype(fp8)
    na8a = q8.reshape(NG, 2, 128, D).transpose(2, 0, 1, 3)       # [128,g,2,D]

    W1v = np.ascontiguousarray(
        (W1 / S3).astype(np.float32).reshape(2, 128, H).transpose(1, 0, 2)
        .reshape(128, 512).astype(bf))
    b1r = np.broadcast_to(np.asarray(b1, dtype=np.float32).astype(bf), (128, H))
    W2v = np.ascontiguousarray(
        np.asarray(W2, dtype=np.float32).reshape(2, 128, M2).transpose(1, 0, 2)
        .reshape(128, 1024).astype(bf))
    b2x = np.asarray(b2, dtype=np.float32).copy()
    b2x[:D] += 1.0  # fold the FiLM (1+gamma) into the bias broadcast
    b2bv = np.ascontiguousarray(np.broadcast_to(b2x, (128, M2)))

    pa_v = np.ascontiguousarray(p_code.reshape(NJT, 128).T)  # [128, 64]

    in_maps = []
    for r in range(NCORES):
        sl = slice(r * R, (r + 1) * R)
        # rotate the j axis so this core's own rows start at local tile 0
        trot = [(t + 8 * r) % NJT for t in range(NJT)]
        grot = [(g + 4 * r) % NG for g in range(NG)]
        cbfv = np.empty((128, 3328), dtype=bf)
        cbfv[:, 0:R] = np.broadcast_to(p_code[sl], (128, R))
        cbfv[:, R:R + 512] = W1v
        cbfv[:, R + 512:R + 1536] = W2v
        cbfv[:, 2560:2560 + H] = b1r
        cbfv[:, 2816:3328] = b2bv.astype(bf)
        cffv = np.ascontiguousarray(pa_v[:, trot].astype(np.float32))
        fnT8v = np.ascontiguousarray(
            fn8T[:, sl].reshape(2, 128, R).transpose(1, 0, 2).reshape(128, 2 * R))
        in_maps.append({
            "xT8": np.ascontiguousarray(xT8a[:, trot]).reshape(128, NJT * 256),
            "na8": np.ascontiguousarray(na8a[:, grot]).reshape(128, NG * 512),
            "fnT8": fnT8v,
            "nodes": np.ascontiguousarray(nodes_s[sl]),
            "cbf": cbfv,
            "cff": cffv,
        })
    return in_maps, order, thresh, temp


def kernel(nodes, patient_indices, threshold, temperature, W1, b1, W2, b2):
    from concourse.bass_utils import run_bass_kernel_spmd

    in_maps, order, thresh, temp = _prep(
        nodes, patient_indices, threshold, temperature, W1, b1, W2, b2)
    nc = _build(thresh, temp)
    res = run_bass_kernel_spmd(nc, in_maps, list(range(NCORES)),
                               trace=bool(int(__import__("os").environ.get("BASS_KERNEL_TRACE", "0"))))
    kernel.last_results = res
    outp = np.concatenate([res.results[i]["out"] for i in range(NCORES)], axis=0)
    unperm = np.empty_like(outp)
    unperm[order] = outp
    return unperm.astype(np.float32)


kernel.last_results = None


# revision 20
# speedup vs baseline: 1.1776x; 1.0426x over previous
"""Distributed Trainium2 kernel for AdaptiveSocialFusion (GNN message passing).

Row-parallel across 8 NeuronCores: each core owns B/8 = 1024 output rows.
The host replicates shared operands to every core (no collectives) and does
layout-only prep: sort rows by patient id, L2-normalize, quantize to fp8-e4m3
in DoubleRow-interleaved layouts.

Per core, fp8 DoubleRow matmuls do both O(B*R*D) products in one pass each:
  sim:  simT[j,i] = sum_d fn8[j,d]*fn8[i,d]    (lhsT = xT8 j-tile, K=256 via DR)
  agg:  wnT[d,i]  = sum_j adj8[j,i]*na8[j,d]   (lhsT = na8 d-chunk, moving = adj8)
Masking happens BEFORE the activation: same-patient sim entries get -1e9 added
(patient-sorted rows confine them to ~6 j-tiles per i-chunk), then one scalar
activation per 2-j-tile group computes adj8 = fp8(K*exp(scale*sim + bias)) --
exp==sigmoid to <1% in the far tail the data lives in, and the K=1024 scaling
(folded into the bias) keeps adj inside fp8's dynamic range. K cancels in the
row-normalization; the gate's tanh absorbs 1/K via its free affine input.
Row-sums are recovered on the vector engine (adj8 tile adds) + one ones-matmul
per i-chunk; the FiLM MLP consumes wnT directly (no transposes anywhere).
"""
import numpy as np

B = 8192
D = 256
H = 256
M2 = 512          # 2*D
NCORES = 8
R = B // NCORES   # 1024 rows per core
NJT = B // 128    # 64 global j-tiles
NG = NJT // 2     # 32 j-groups (2 tiles per activation / DoubleRow pair)
NIC = 2           # i-chunks of 512
IC = 512
S = 32.0          # fp8 scale for normalized features (both sim operands)
S3 = 16.0         # fp8 scale for raw nodes (agg stationary)
KADJ = 1024.0     # adjacency pre-scale folded into the exp bias


def _build(thresh: float, temp: float):
    import concourse.bass as bass
    import concourse.tile as tile
    from concourse import bacc, mybir

    f32 = mybir.dt.float32
    bf16 = mybir.dt.bfloat16
    f8 = mybir.dt.float8e4
    AF = mybir.ActivationFunctionType
    ALU = mybir.AluOpType
    DR = mybir.MatmulPerfMode.DoubleRow

    nc = bacc.Bacc("TRN2", target_bir_lowering=False, debug=False, num_devices=NCORES)

    xT8 = nc.declare_dram_parameter("xT8", [128, NJT * 256], f8, isOutput=False)
    na8 = nc.declare_dram_parameter("na8", [128, NG * 512], f8, isOutput=False)
    fnT8 = nc.declare_dram_parameter("fnT8", [128, 2 * R], f8, isOutput=False)
    nodes = nc.declare_dram_parameter("nodes", [R, D], f32, isOutput=False)
    cbf = nc.declare_dram_parameter("cbf", [128, 3328], bf16, isOutput=False)
    cff = nc.declare_dram_parameter("cff", [128, 64], f32, isOutput=False)
    out = nc.declare_dram_parameter("out", [R, D], f32, isOutput=True)

    act_scale = temp / (S * S)
    act_bias = float(np.log(KADJ)) - temp * thresh

    with tile.TileContext(nc) as tc:
        with (
            tc.tile_pool(name="const", bufs=1) as cpool,
            tc.tile_pool(name="resident", bufs=1) as rpool,
            tc.tile_pool(name="rot", bufs=3) as rot,
            tc.tile_pool(name="vrot", bufs=2) as vrot,
            tc.tile_pool(name="simp", bufs=2, space="PSUM") as simp,
            tc.tile_pool(name="wnp", bufs=1, space="PSUM") as wnp,
            tc.tile_pool(name="tailp", bufs=1, space="PSUM") as tailp,
        ):
            # ---- tiny warmup to pull the exp/tanh ACT table load off the
            # critical path (it runs during the DMA lead-in)
            wu = cpool.tile([1, 1], f32, tag="wu", name="wu")
            nc.vector.memset(wu[:], 0.0)
            wu2 = cpool.tile([1, 1], f32, tag="wu2", name="wu2")
            nc.scalar.activation(wu2[:], wu[:], AF.Exp)

            abias_sb = cpool.tile([128, 1], f32, tag="abias", name="abias")
            nc.vector.memset(abias_sb[:], act_bias)
            ascale_sb = cpool.tile([128, 1], f32, tag="ascale", name="ascale")
            nc.vector.memset(ascale_sb[:], act_scale)
            gscale_sb = cpool.tile([128, 1], f32, tag="gscale", name="gscale")
            nc.vector.memset(gscale_sb[:], -2.0 / KADJ)
            # DoubleRow weight APs need 16B-aligned pair stride: [128,2,16] pad
            ones8 = cpool.tile([128, 32], f8, tag="ones8", name="ones8")
            nc.vector.memset(ones8[:], 1.0)
            ones8_v = ones8[:].rearrange("p (two x) -> p two x", two=2)[:, :, 0:1]
            ones_f = cpool.tile([1, 128], f32, tag="ones_f", name="ones_f")
            nc.vector.memset(ones_f[:], 1.0)

            # ---- streamed inputs: many small pieces, first-needed first
            fnT_sb = rpool.tile([128, 2 * R], f8, tag="fnT", name="fnT")
            xT_sb = rpool.tile([128, NJT * 256], f8, tag="xT", name="xT")
            na_sb = rpool.tile([128, NG * 512], f8, tag="na", name="na")
            cbf_sb = cpool.tile([128, 3328], bf16, tag="cbf", name="cbf")
            cff_sb = cpool.tile([128, 64], f32, tag="cff", name="cff")
            nc.sync.dma_start(fnT_sb[:, 0:512], fnT8[:, 0:512])
            nc.gpsimd.dma_start(fnT_sb[:, 1024:1536], fnT8[:, 1024:1536])
            nc.sync.dma_start(xT_sb[:, 0:512], xT8[:, 0:512])
            nc.gpsimd.dma_start(na_sb[:, 0:512], na8[:, 0:512])
            nc.sync.dma_start(cff_sb[:], cff[:, :])                  # pa codes
            nc.gpsimd.dma_start(cbf_sb[:, 0:512], cbf[:, 0:512])     # pb ic0
            nc.sync.dma_start(fnT_sb[:, 512:1024], fnT8[:, 512:1024])
            nc.gpsimd.dma_start(fnT_sb[:, 1536:2048], fnT8[:, 1536:2048])
            nc.sync.dma_start(xT_sb[:, 512:1024], xT8[:, 512:1024])
            nc.gpsimd.dma_start(na_sb[:, 512:1024], na8[:, 512:1024])
            nc.gpsimd.dma_start(cbf_sb[:, 512:1024], cbf[:, 512:1024])
            nc.sync.dma_start(cbf_sb[:, 2816:3328], cbf[:, 2816:3328])  # b2b
            nodes_sb = [rpool.tile([128, D], f32, tag=f"nodes{t}", name=f"nodes{t}")
                        for t in range(8)]
            for k in range(1, 16):
                nc.sync.dma_start(xT_sb[:, k * 1024:(k + 1) * 1024],
                                  xT8[:, k * 1024:(k + 1) * 1024])
                nc.gpsimd.dma_start(na_sb[:, k * 1024:(k + 1) * 1024],
                                    na8[:, k * 1024:(k + 1) * 1024])
                if k == 4:
                    nc.sync.dma_start(cbf_sb[:, 1024:1920], cbf[:, 1024:1920])
                    nc.gpsimd.dma_start(cbf_sb[:, 1920:2816], cbf[:, 1920:2816])
                if 8 <= k < 12:
                    t = 2 * (k - 8)
                    nc.sync.dma_start(nodes_sb[t][:], nodes[t * 128:(t + 1) * 128, :])
                    nc.gpsimd.dma_start(nodes_sb[t + 1][:],
                                        nodes[(t + 1) * 128:(t + 2) * 128, :])

            pb_sb = cbf_sb[:, 0:R]                     # local i codes (bcast)
            w1_sb = cbf_sb[:, R:R + 512]               # [dc*256 + h]
            w2_sb = cbf_sb[:, R + 512:R + 1536]        # [hc*512 + d2]
            b1r_sb = cbf_sb[:, 2560:2560 + H]          # b1 as a row (rank-1 fold)
            b2b_sb = cbf_sb[:, 2816:3328]              # b2 bcast, gamma half +1
            pa_sb = cff_sb[:, 0:NJT]                   # j-tile codes (f32)

            def xT_lhsT(jt):
                return xT_sb[:, jt * 256:(jt + 1) * 256].rearrange(
                    "p (two j) -> p two j", two=2)

            def na_lhsT(g, c):
                v = na_sb[:, g * 512:(g + 1) * 512].rearrange(
                    "p (two d) -> p two d", two=2)
                return v[:, :, c * 128:(c + 1) * 128]

            fnT_v = fnT_sb[:].rearrange("p (two i) -> p two i", two=2)

            # Each core's xT8/na8/pa inputs are rotated by the host so its own
            # rows start at local j-tile 0; same-patient pairs then live at
            # FIXED local tiles [4*ic-1, 4*ic+5) mod 64 (patient-sorted,
            # groups <= 128), letting one SPMD program serve all cores.
            def masked_tiles(ic):
                return set((4 * ic + k - 1) % NJT for k in range(6))

            wn_ps = {}

            def main_group(ic, g, mtiles):
                sim_ps = simp.tile([128, 1024], f32, tag="sim", name="sim")
                for half in range(2):
                    jt = 2 * g + half
                    nc.tensor.matmul(sim_ps[:, half * IC:(half + 1) * IC],
                                     xT_lhsT(jt),
                                     fnT_v[:, :, ic * IC:(ic + 1) * IC],
                                     start=True, stop=True, perf_mode=DR)
                for half in range(2):
                    jt = 2 * g + half
                    if jt in mtiles:
                        eqb = vrot.tile([128, IC], f32, tag="eqb", name="eqb")
                        nc.vector.tensor_scalar(
                            eqb[:], pb_sb[:, ic * IC:(ic + 1) * IC],
                            pa_sb[:, jt:jt + 1], -1e9,
                            op0=ALU.is_equal, op1=ALU.mult)
                        sl = sim_ps[:, half * IC:(half + 1) * IC]
                        nc.vector.tensor_add(sl, sl, eqb[:])
                adj8 = rot.tile([128, 1024], f8, tag="adj", name="adj")
                nc.scalar.activation(adj8[:], sim_ps[:], AF.Exp,
                                     bias=abias_sb[:], scale=ascale_sb[:])
                adj_v = adj8[:].rearrange("p (two i) -> p two i", two=2)
                for c in range(2):
                    nc.tensor.matmul(wn_ps[c][:],
                                     na_lhsT(g, c), adj_v,
                                     start=(g == 0), stop=(g == NG - 1),
                                     perf_mode=DR)
                nc.tensor.matmul(rs_ps[:], ones8_v, adj_v,
                                 start=(g == 0), stop=(g == NG - 1),
                                 perf_mode=DR)

            def tail_pre(ic):
                rskp = vrot.tile([1, IC], f32, tag="rskp", name="rskp")
                nc.vector.tensor_scalar_add(rskp[:], rs_ps[:], KADJ * 1e-6)
                rskb = vrot.tile([1, IC], bf16, tag="rskb", name="rskb")
                nc.vector.tensor_copy(rskb[:], rskp[:])
                wnn = []
                for c in range(2):
                    w = rot.tile([128, IC], bf16, tag=f"wnn{c}", name=f"wnn{c}")
                    nc.vector.tensor_copy(w[:], wn_ps[c][:])
                    wnn.append(w)
                return rskp, rskb, wnn

            def tail_post(ic, rskp, rskb, wnn):
                # rs moved to partitions via 4 K=1 matmuls
                gate_ps = tailp.tile([128, 4], f32, tag="mlp", name="gate_ps")
                for m in range(4):
                    nc.tensor.matmul(gate_ps[:, m:m + 1],
                                     rskp[0:1, m * 128:(m + 1) * 128],
                                     ones_f[0:1, 0:1])
                rcp4 = vrot.tile([128, 4], f32, tag="rcp4", name="rcp4")
                nc.vector.reciprocal(rcp4[:], gate_ps[:])
                # tanh(y) = 1 - 2u/(1+u), u = exp(-2y): no tanh table needed
                u_sb = vrot.tile([128, 4], f32, tag="gate_u", name="gate_u")
                nc.scalar.activation(u_sb[:], gate_ps[:], AF.Exp,
                                     scale=gscale_sb[:])
                d_sb = vrot.tile([128, 4], f32, tag="gate_d", name="gate_d")
                nc.vector.tensor_scalar_add(d_sb[:], u_sb[:], 1.0)
                r4g = vrot.tile([128, 4], f32, tag="gate_r", name="gate_r")
                nc.vector.reciprocal(r4g[:], d_sb[:])
                nc.vector.tensor_mul(u_sb[:], u_sb[:], r4g[:])
                gate_sb = vrot.tile([128, 4], f32, tag="gate", name="gate")
                nc.vector.tensor_scalar(gate_sb[:], u_sb[:], -2.0, 1.0,
                                        op0=ALU.mult, op1=ALU.add)
                rg = vrot.tile([128, 4], f32, tag="rg", name="rg")
                nc.vector.tensor_mul(rg[:], rcp4[:], gate_sb[:])
                g2 = []
                for m in range(4):
                    g2m = vrot.tile([128, M2], bf16, tag=f"g2{m % 2}",
                                    name=f"g2{m % 2}")
                    nc.vector.tensor_scalar_mul(g2m[:], b2b_sb[:],
                                                gate_sb[:, m:m + 1])
                    g2.append(g2m)
                # FiLM MLP on UNNORMALIZED wnT with b1*rs rank-1 term;
                # the 1/rs scaling commutes past the relu to the f output
                h_sb = []
                for hc in range(2):
                    h_ps = tailp.tile([128, IC], f32, tag="mlp", name="h_ps")
                    for dc in range(2):
                        nc.tensor.matmul(
                            h_ps[:],
                            w1_sb[:, dc * 256 + hc * 128:dc * 256 + (hc + 1) * 128],
                            wnn[dc][:], start=(dc == 0), stop=False)
                    nc.tensor.matmul(h_ps[:],
                                     b1r_sb[0:1, hc * 128:(hc + 1) * 128],
                                     rskb[0:1, :], start=False, stop=True)
                    hs = rot.tile([128, IC], bf16, tag=f"h{hc}", name=f"h{hc}")
                    nc.vector.tensor_scalar_max(hs[:], h_ps[:], 0.0)
                    h_sb.append(hs)
                fpool = simp if ic == NIC - 1 else tailp
                ftag = "sim" if ic == NIC - 1 else "mlp"
                for m in range(4):
                    it = ic * 4 + m
                    f_ps = fpool.tile([128, M2], f32, tag=ftag, name="f_ps")
                    for hc in range(2):
                        nc.tensor.matmul(
                            f_ps[:], h_sb[hc][:, m * 128:(m + 1) * 128],
                            w2_sb[:, hc * M2:(hc + 1) * M2],
                            start=(hc == 0), stop=(hc == 1))
                    t_sb = vrot.tile([128, M2], bf16, tag=f"tcmb{m % 2}",
                                     name=f"tcmb{m % 2}")
                    nc.vector.tensor_scalar_mul(t_sb[:], f_ps[:], rg[:, m:m + 1])
                    nc.vector.tensor_add(t_sb[:], t_sb[:], g2[m][:])
                    ob = vrot.tile([128, D], f32, tag=f"ob{m % 2}",
                                   name=f"ob{m % 2}")
                    nt = nodes_sb[it]
                    nc.vector.tensor_mul(ob[:], t_sb[:, 0:D], nt[:])
                    nc.vector.tensor_add(ob[:], ob[:], nt[:])
                    nc.vector.tensor_add(ob[:], ob[:], t_sb[:, D:M2])
                    half = D // 2
                    nc.sync.dma_start(out[it * 128:(it + 1) * 128, 0:half],
                                      ob[:, 0:half])
                    nc.gpsimd.dma_start(out[it * 128:(it + 1) * 128, half:D],
                                        ob[:, half:D])

            pend = None
            for ic in range(NIC):
                for c in range(2):
                    wn_ps[c] = wnp.tile([128, IC], f32, tag=f"wn{c}",
                                        name=f"wn{c}")
                rs_ps = wnp.tile([1, IC], f32, tag="rs", name="rs")
                mt = masked_tiles(ic)
                for g in range(NG):
                    main_group(ic, g, mt)
                    if g == 7 and pend is not None:
                        tail_post(*pend)
                        pend = None
                pend = (ic,) + tail_pre(ic)
            tail_post(*pend)

    nc.compile()
    return nc


def _prep(nodes, patient_indices, threshold, temperature, W1, b1, W2, b2):
    """Host-side layout prep. Returns (in_maps, order, thresh, temp)."""
    import ml_dtypes

    fp8 = ml_dtypes.float8_e4m3
    bf = ml_dtypes.bfloat16

    thresh = float(np.clip(np.asarray(threshold, dtype=np.float64)[0], 0.0, 0.99))
    temp = float(np.asarray(temperature, dtype=np.float64)[0])

    nodes = np.asarray(nodes, dtype=np.float32)
    assert nodes.shape == (B, D), f"kernel hardcodes B={B}, D={D}; got {nodes.shape}"
    # Sort rows by patient so same-patient pairs live near the diagonal;
    # unpermute the output at the end.
    p_int = np.asarray(patient_indices).astype(np.int64)
    order = np.argsort(p_int, kind="stable")
    nodes_s = np.ascontiguousarray(nodes[order])
    p_s = p_int[order]
    _, inv = np.unique(p_s, return_inverse=True)
    assert np.bincount(inv).max() <= 128, "patient group exceeds diagonal window"
    codes = (np.arange(inv.max() + 1, dtype=np.uint16) + 0x0100).view(bf)
    p_code = codes[inv]  # [B] bf16, distinct value per patient class

    norm = np.maximum(np.linalg.norm(nodes_s, axis=1, keepdims=True), 1e-12)
    fn8 = (S * nodes_s / norm).astype(fp8)             # [B, D]
    fn8T = np.ascontiguousarray(fn8.T)                 # [D, B]
    # xT8: [p, jt, ko, j] -- DoubleRow stationary pairs over d
    xT8a = fn8T.reshape(2, 128, NJT, 128).transpose(1, 2, 0, 3)  # [128,jt,2,128]
    # na8: [p, g, ko, d] -- DoubleRow stationary pairs over j (2 tiles/group)
    q8 = (S3 * nodes_s).astype(fp8)
    na8a = q8.reshape(NG, 2, 128, D).transpose(2, 0, 1, 3)       # [128,g,2,D]

    W1v = np.ascontiguousarray(
        (W1 / S3).astype(np.float32).reshape(2, 128, H).transpose(1, 0, 2)
        .reshape(128, 512).astype(bf))
    b1r = np.broadcast_to(np.asarray(b1, dtype=np.float32).astype(bf), (128, H))
    W2v = np.ascontiguousarray(
        np.asarray(W2, dtype=np.float32).reshape(2, 128, M2).transpose(1, 0, 2)
        .reshape(128, 1024).astype(bf))
    b2x = np.asarray(b2, dtype=np.float32).copy()
    b2x[:D] += 1.0  # fold the FiLM (1+gamma) into the bias broadcast
    b2bv = np.ascontiguousarray(np.broadcast_to(b2x, (128, M2)))

    pa_v = np.ascontiguousarray(p_code.reshape(NJT, 128).T)  # [128, 64]

    in_maps = []
    for r in range(NCORES):
        sl = slice(r * R, (r + 1) * R)
        # rotate the j axis so this core's own rows start at local tile 0
        trot = [(t + 8 * r) % NJT for t in range(NJT)]
        grot = [(g + 4 * r) % NG for g in range(NG)]
        cbfv = np.empty((128, 3328), dtype=bf)
        cbfv[:, 0:R] = np.broadcast_to(p_code[sl], (128, R))
        cbfv[:, R:R + 512] = W1v
        cbfv[:, R + 512:R + 1536] = W2v
        cbfv[:, 2560:2560 + H] = b1r
        cbfv[:, 2816:3328] = b2bv.astype(bf)
        cffv = np.ascontiguousarray(pa_v[:, trot].astype(np.float32))
        fnT8v = np.ascontiguousarray(
            fn8T[:, sl].reshape(2, 128, R).transpose(1, 0, 2).reshape(128, 2 * R))
        in_maps.append({
            "xT8": np.ascontiguousarray(xT8a[:, trot]).reshape(128, NJT * 256),
            "na8": np.ascontiguousarray(na8a[:, grot]).reshape(128, NG * 512),
            "fnT8": fnT8v,
            "nodes": np.ascontiguousarray(nodes_s[sl]),
            "cbf": cbfv,
            "cff": cffv,
        })
    return in_maps, order, thresh, temp


def kernel(nodes, patient_indices, threshold, temperature, W1, b1, W2, b2):
    from concourse.bass_utils import run_bass_kernel_spmd

    in_maps, order, thresh, temp = _prep(
        nodes, patient_indices, threshold, temperature, W1, b1, W2, b2)
    nc = _build(thresh, temp)
    res = run_bass_kernel_spmd(nc, in_maps, list(range(NCORES)),
                               trace=bool(int(__import__("os").environ.get("BASS_KERNEL_TRACE", "0"))))
    kernel.last_results = res
    outp = np.concatenate([res.results[i]["out"] for i in range(NCORES)], axis=0)
    unperm = np.empty_like(outp)
    unperm[order] = outp
    return unperm.astype(np.float32)


kernel.last_results = None
